# revision 10
# baseline (speedup 1.0000x reference)
"""Trainium2 Bass kernel for y = x @ W^T + b  (4096x4096 @ 4096x4096 + 4096).

Sharding: data-parallel over batch, R=8 groups. Core r computes
yT_r = W @ x_r^T + b[:, None]  ([4096, 512], output transposed) and the
host reassembles y. No collectives.

All layout work happens on the host: x and W are transposed, tiled to
the exact SBUF layout, and cast to bf16 in numpy. The device kernel is
nothing but back-to-back bf16 matmuls (fp32 PSUM accumulate):

  - xT_r [128, 32*512] bf16 (4MB) resident in SBUF, DMA'd in graduated
    chunks (small first so compute starts early).
  - Prologue: the first 2 o-tiles' accumulations run chunk-major so the
    PE saturates while x is still arriving.
  - Steady state per o-tile (32): W slab [128, 32*128] bf16 DMA
    (triple-buffered, 147 GB/s sustained), 32 k-tile matmuls (N=512)
    accumulating in one PSUM bank, ScalarE eviction fused with bias
    add, HWDGE DMA out.

PE roofline: 1024 MM x 512 cols / 2.4 GHz = 218.5 us per core.
"""

import os
import sys

for _p in ("/opt/trn_rl_repo", "/opt/pypackages"):
    if _p not in sys.path and os.path.isdir(_p):
        sys.path.append(_p)

import numpy as np
import ml_dtypes

import concourse.bass as bass
import concourse.tile as tile
from concourse import bacc, mybir
from concourse.bass_utils import run_bass_kernel_spmd

N_CORES = 8
R = 8                          # batch groups
BATCH = 4096
IN_F = 4096
OUT_F = 4096
P = 128
BR = BATCH // R                # 512 batch rows per core
KT = IN_F // P                 # 32 contraction tiles
OT = OUT_F // P                # 32 output-feature tiles per core
# x chunk schedule in k-tiles (1 k-tile = 128KB here)
XCHUNKS = [(0, 2), (2, 2), (4, 4), (8, 8), (16, 8), (24, 8)]
NPRE = 2                       # o-tiles interleaved in the prologue

_F32 = mybir.dt.float32
_BF16 = mybir.dt.bfloat16
_BF16_NP = ml_dtypes.bfloat16

_compiled_nc = None


def _build():
    nc = bacc.Bacc("TRN2", target_bir_lowering=False, debug=False,
                   num_devices=N_CORES)

    # Host-pretiled layouts (see _prep_inputs):
    #   xt[p, it*BR + b]            = x_r[b, it*128 + p]            (bf16)
    #   wt[ot*128 + p, it*128 + o2] = w[ot*128 + o2, it*128 + p]    (bf16)
    #   bias_t[p, ot]               = b[ot*128 + p]                 (f32)
    xt = nc.dram_tensor("xt", [P, KT * BR], _BF16, kind="ExternalInput")
    wt = nc.dram_tensor("wt", [OT * P, KT * P], _BF16, kind="ExternalInput")
    bias = nc.dram_tensor("bias", [P, OT], _F32, kind="ExternalInput")
    out = nc.dram_tensor("out", [OUT_F, BR], _F32, kind="ExternalOutput")

    with tile.TileContext(nc) as tc:
        with tc.tile_pool(name="const", bufs=1) as const, \
             tc.tile_pool(name="wslab", bufs=3) as wpool, \
             tc.tile_pool(name="psum", bufs=4, space="PSUM") as pspool, \
             tc.tile_pool(name="yout", bufs=3) as ypool:

            bias_sb = const.tile([P, OT], _F32)
            nc.scalar.dma_start(out=bias_sb[:], in_=bias[:, :])

            # PE warm-up fuel: a junk tile whose first 16 cols come from the
            # bias DMA, so matmuls on it cannot start before ~10us -- after
            # first_useful, inside the PE-idle window before chunk0 lands.
            dummy = const.tile([P, 528], _BF16)
            nc.vector.memset(dummy[:, 16:], 1.0)
            nc.vector.tensor_copy(out=dummy[:, :16], in_=bias_sb[:, :16])

            ps_junk = pspool.tile([P, BR], _F32, name="psjunk", tag="ps")

            def junk_mms(n):
                for i in range(n):
                    nc.tensor.matmul(ps_junk[:], lhsT=dummy[:, 0:P],
                                     rhs=dummy[:, 16:16 + BR],
                                     start=(i == 0), stop=(i == n - 1))

            # ---- W slabs for the prologue o-tiles
            w_pre = []
            for ot in range(NPRE):
                w_sb = wpool.tile([P, KT * P], _BF16, name=f"w{ot}", tag="w")
                nc.sync.dma_start(out=w_sb[:],
                                  in_=wt[ot * P:(ot + 1) * P, :])
                w_pre.append(w_sb)

            # ---- x on the gpsimd (SWDGE) ring
            x_sb = const.tile([P, KT * BR], _BF16)
            for it0, nit in XCHUNKS:
                nc.gpsimd.dma_start(
                    out=x_sb[:, it0 * BR:(it0 + nit) * BR],
                    in_=xt[:, it0 * BR:(it0 + nit) * BR])

            # ~3.5us of junk matmuls: release the HAM clock throttle while
            # the PE would otherwise idle waiting for the first x chunk.
            junk_mms(8)

            def evict(ot, y_sb, ps):
                nc.scalar.activation(y_sb[:], ps[:],
                                     mybir.ActivationFunctionType.Identity,
                                     bias=bias_sb[:, ot:ot + 1])
                nc.scalar.dma_start(out=out[ot * P:(ot + 1) * P, :],
                                    in_=y_sb[:])

            # ---- prologue: first NPRE o-tiles chunk-major over x arrival.
            # A few junk matmuls after the small early chunks bridge the
            # delivery gaps so the warm clock never re-throttles.
            ps_pre = [pspool.tile([P, BR], _F32, name=f"psp{ot}", tag="ps")
                      for ot in range(NPRE)]
            for ci, (it0, nit) in enumerate(XCHUNKS):
                for ot in range(NPRE):
                    for l in range(nit):
                        it = it0 + l
                        nc.tensor.matmul(
                            ps_pre[ot][:],
                            lhsT=w_pre[ot][:, it * P:(it + 1) * P],
                            rhs=x_sb[:, it * BR:(it + 1) * BR],
                            start=(it == 0), stop=(it == KT - 1))
                if ci in (0, 1, 2):
                    junk_mms(4)
            for ot in range(NPRE):
                y_sb = ypool.tile([P, BR], _F32, name=f"y{ot}", tag="y")
                evict(ot, y_sb, ps_pre[ot])

            # ---- steady state over the remaining o-tiles
            for ot in range(NPRE, OT):
                w_sb = wpool.tile([P, KT * P], _BF16, name=f"w{ot}", tag="w")
                nc.sync.dma_start(out=w_sb[:],
                                  in_=wt[ot * P:(ot + 1) * P, :])

                y_sb = ypool.tile([P, BR], _F32, name=f"y{ot}", tag="y")
                ps = pspool.tile([P, BR], _F32, name=f"ps{ot}", tag="ps")
                for it in range(KT):
                    nc.tensor.matmul(ps[:],
                                     lhsT=w_sb[:, it * P:(it + 1) * P],
                                     rhs=x_sb[:, it * BR:(it + 1) * BR],
                                     start=(it == 0), stop=(it == KT - 1))
                evict(ot, y_sb, ps)

    nc.compile()
    return nc


def _get_nc():
    global _compiled_nc
    if _compiled_nc is None:
        _compiled_nc = _build()
    return _compiled_nc


def _prep_inputs(inputs):
    x = np.ascontiguousarray(np.asarray(inputs["x"], dtype=np.float32))
    w = np.ascontiguousarray(np.asarray(inputs["weight"], dtype=np.float32))
    b = np.ascontiguousarray(np.asarray(inputs["bias"], dtype=np.float32))

    # x tiles per batch group r: [p, it*BR + b] = x_r[b, it*128 + p]
    xts = []
    for r in range(R):
        xs = x[r * BR:(r + 1) * BR, :]                      # [BR, IN_F]
        xt = xs.T.reshape(KT, P, BR).transpose(1, 0, 2)     # [P, KT, BR]
        xts.append(np.ascontiguousarray(
            xt.astype(_BF16_NP).reshape(P, KT * BR)))

    # W tiles: [ot*128 + p, it*128 + o2] = w[ot*128 + o2, it*128 + p]
    wtt = w.T.reshape(KT, P, OT, P).transpose(2, 1, 0, 3)   # [OT,P,KT,P]
    wt = np.ascontiguousarray(wtt.astype(_BF16_NP).reshape(OT * P, KT * P))
    bias_t = np.ascontiguousarray(b.reshape(OT, P).T)       # [P, OT]

    return [{"xt": xts[r], "wt": wt, "bias": bias_t} for r in range(R)]


def _run(inputs, trace=False, trace_cores=None):
    nc = _get_nc()
    in_maps = _prep_inputs(inputs)
    res = run_bass_kernel_spmd(nc, in_maps, core_ids=list(range(N_CORES)),
                               trace=trace, trace_cores=trace_cores)
    y = np.empty((BATCH, OUT_F), dtype=np.float32)
    for r in range(R):
        y[r * BR:(r + 1) * BR, :] = res.results[r]["out"].T
    return y, res


def kernel(**inputs):
    y, _ = _run(inputs)
    return y



# revision 11
# speedup vs baseline: 1.0134x; 1.0134x over previous
"""Trainium2 Bass kernel for y = x @ W^T + b  (4096x4096 @ 4096x4096 + 4096).

Sharding: data-parallel over batch, R=8 groups. Core r computes
yT_r = W @ x_r^T + b[:, None]  ([4096, 512], output transposed) and the
host reassembles y. No collectives.

All layout work happens on the host: x and W are transposed, tiled to
the exact SBUF layout, and cast to bf16 in numpy. The device kernel is
nothing but back-to-back bf16 matmuls (fp32 PSUM accumulate):

  - xT_r [128, 32*512] bf16 (4MB) resident in SBUF, DMA'd in graduated
    chunks (small first so compute starts early).
  - Prologue: the first 2 o-tiles' accumulations run chunk-major so the
    PE saturates while x is still arriving.
  - Steady state per o-tile (32): W slab [128, 32*128] bf16 DMA
    (triple-buffered, 147 GB/s sustained), 32 k-tile matmuls (N=512)
    accumulating in one PSUM bank, ScalarE eviction fused with bias
    add, HWDGE DMA out.

PE roofline: 1024 MM x 512 cols / 2.4 GHz = 218.5 us per core.
"""

import os
import sys

for _p in ("/opt/trn_rl_repo", "/opt/pypackages"):
    if _p not in sys.path and os.path.isdir(_p):
        sys.path.append(_p)

import numpy as np
import ml_dtypes

import concourse.bass as bass
import concourse.tile as tile
from concourse import bacc, mybir
from concourse.bass_utils import run_bass_kernel_spmd

N_CORES = 8
R = 8                          # batch groups
BATCH = 4096
IN_F = 4096
OUT_F = 4096
P = 128
BR = BATCH // R                # 512 batch rows per core
KT = IN_F // P                 # 32 contraction tiles
OT = OUT_F // P                # 32 output-feature tiles per core
# x chunk schedule in k-tiles (1 k-tile = 128KB here). Finer chunks
# than v1's (8,8)/(16,8)/(24,8) tail: the SWDGE stream rate is
# unchanged (measured), but prologue matmuls unblock per 4-k-tile
# granule instead of waiting for a whole 1MB chunk, pulling the end of
# the matmul stream in by ~2us.
XCHUNKS = [(0, 2), (2, 2), (4, 4), (8, 4), (12, 4), (16, 4), (20, 4),
           (24, 4), (28, 4)]
NPRE = 2                       # o-tiles interleaved in the prologue

_F32 = mybir.dt.float32
_BF16 = mybir.dt.bfloat16
_BF16_NP = ml_dtypes.bfloat16

_compiled_nc = None


def _build():
    nc = bacc.Bacc("TRN2", target_bir_lowering=False, debug=False,
                   num_devices=N_CORES)

    # Host-pretiled layouts (see _prep_inputs):
    #   xt[p, it*BR + b]            = x_r[b, it*128 + p]            (bf16)
    #   wt[ot*128 + p, it*128 + o2] = w[ot*128 + o2, it*128 + p]    (bf16)
    #   bias_t[p, ot]               = b[ot*128 + p]                 (f32)
    xt = nc.dram_tensor("xt", [P, KT * BR], _BF16, kind="ExternalInput")
    wt = nc.dram_tensor("wt", [OT * P, KT * P], _BF16, kind="ExternalInput")
    bias = nc.dram_tensor("bias", [P, OT], _F32, kind="ExternalInput")
    out = nc.dram_tensor("out", [OUT_F, BR], _F32, kind="ExternalOutput")

    with tile.TileContext(nc) as tc:
        with tc.tile_pool(name="const", bufs=1) as const, \
             tc.tile_pool(name="wslab", bufs=3) as wpool, \
             tc.tile_pool(name="psum", bufs=4, space="PSUM") as pspool, \
             tc.tile_pool(name="yout", bufs=3) as ypool:

            bias_sb = const.tile([P, OT], _F32)
            nc.scalar.dma_start(out=bias_sb[:], in_=bias[:, :])

            # PE warm-up fuel: a junk tile whose first 16 cols come from the
            # bias DMA, so matmuls on it cannot start before ~10us -- after
            # first_useful, inside the PE-idle window before chunk0 lands.
            dummy = const.tile([P, 528], _BF16)
            nc.vector.memset(dummy[:, 16:], 1.0)
            nc.vector.tensor_copy(out=dummy[:, :16], in_=bias_sb[:, :16])

            ps_junk = pspool.tile([P, BR], _F32, name="psjunk", tag="ps")

            def junk_mms(n):
                for i in range(n):
                    nc.tensor.matmul(ps_junk[:], lhsT=dummy[:, 0:P],
                                     rhs=dummy[:, 16:16 + BR],
                                     start=(i == 0), stop=(i == n - 1))

            # ---- W slabs for the prologue o-tiles, each as two 512KB
            # halves (4KB rows, near line-rate): o1's first half lands
            # ~16.6us so its chunk-major matmuls don't stall until the
            # full slab at ~22.5us (v1's one mid-prologue PE gap).
            w_pre = []
            for ot in range(NPRE):
                w_sb = wpool.tile([P, KT * P], _BF16, name=f"w{ot}", tag="w")
                half = KT // 2
                nc.sync.dma_start(
                    out=w_sb[:, 0:half * P],
                    in_=wt[ot * P:(ot + 1) * P, 0:half * P])
                nc.sync.dma_start(
                    out=w_sb[:, half * P:KT * P],
                    in_=wt[ot * P:(ot + 1) * P, half * P:KT * P])
                w_pre.append(w_sb)

            # ---- x on the gpsimd (SWDGE) ring
            x_sb = const.tile([P, KT * BR], _BF16)
            for it0, nit in XCHUNKS:
                nc.gpsimd.dma_start(
                    out=x_sb[:, it0 * BR:(it0 + nit) * BR],
                    in_=xt[:, it0 * BR:(it0 + nit) * BR])

            # ~3.5us of junk matmuls: release the HAM clock throttle while
            # the PE would otherwise idle waiting for the first x chunk.
            junk_mms(8)

            def evict(ot, y_sb, ps):
                nc.scalar.activation(y_sb[:], ps[:],
                                     mybir.ActivationFunctionType.Identity,
                                     bias=bias_sb[:, ot:ot + 1])
                nc.scalar.dma_start(out=out[ot * P:(ot + 1) * P, :],
                                    in_=y_sb[:])

            # ---- prologue: first NPRE o-tiles chunk-major over x arrival.
            # A few junk matmuls after the small early chunks bridge the
            # delivery gaps so the warm clock never re-throttles.
            ps_pre = [pspool.tile([P, BR], _F32, name=f"psp{ot}", tag="ps")
                      for ot in range(NPRE)]
            for ci, (it0, nit) in enumerate(XCHUNKS):
                for ot in range(NPRE):
                    for l in range(nit):
                        it = it0 + l
                        nc.tensor.matmul(
                            ps_pre[ot][:],
                            lhsT=w_pre[ot][:, it * P:(it + 1) * P],
                            rhs=x_sb[:, it * BR:(it + 1) * BR],
                            start=(it == 0), stop=(it == KT - 1))
                if ci in (0, 1, 2):
                    junk_mms(4)
                elif ci == 3:
                    junk_mms(2)
            for ot in range(NPRE):
                y_sb = ypool.tile([P, BR], _F32, name=f"y{ot}", tag="y")
                evict(ot, y_sb, ps_pre[ot])

            # ---- steady state over the remaining o-tiles
            for ot in range(NPRE, OT):
                w_sb = wpool.tile([P, KT * P], _BF16, name=f"w{ot}", tag="w")
                nc.sync.dma_start(out=w_sb[:],
                                  in_=wt[ot * P:(ot + 1) * P, :])

                y_sb = ypool.tile([P, BR], _F32, name=f"y{ot}", tag="y")
                ps = pspool.tile([P, BR], _F32, name=f"ps{ot}", tag="ps")
                for it in range(KT):
                    nc.tensor.matmul(ps[:],
                                     lhsT=w_sb[:, it * P:(it + 1) * P],
                                     rhs=x_sb[:, it * BR:(it + 1) * BR],
                                     start=(it == 0), stop=(it == KT - 1))
                evict(ot, y_sb, ps)

    nc.compile()
    return nc


def _get_nc():
    global _compiled_nc
    if _compiled_nc is None:
        _compiled_nc = _build()
    return _compiled_nc


def _prep_inputs(inputs):
    x = np.ascontiguousarray(np.asarray(inputs["x"], dtype=np.float32))
    w = np.ascontiguousarray(np.asarray(inputs["weight"], dtype=np.float32))
    b = np.ascontiguousarray(np.asarray(inputs["bias"], dtype=np.float32))

    # x tiles per batch group r: [p, it*BR + b] = x_r[b, it*128 + p]
    xts = []
    for r in range(R):
        xs = x[r * BR:(r + 1) * BR, :]                      # [BR, IN_F]
        xt = xs.T.reshape(KT, P, BR).transpose(1, 0, 2)     # [P, KT, BR]
        xts.append(np.ascontiguousarray(
            xt.astype(_BF16_NP).reshape(P, KT * BR)))

    # W tiles: [ot*128 + p, it*128 + o2] = w[ot*128 + o2, it*128 + p]
    wtt = w.T.reshape(KT, P, OT, P).transpose(2, 1, 0, 3)   # [OT,P,KT,P]
    wt = np.ascontiguousarray(wtt.astype(_BF16_NP).reshape(OT * P, KT * P))
    bias_t = np.ascontiguousarray(b.reshape(OT, P).T)       # [P, OT]

    return [{"xt": xts[r], "wt": wt, "bias": bias_t} for r in range(R)]


def _run(inputs, trace=False, trace_cores=None):
    nc = _get_nc()
    in_maps = _prep_inputs(inputs)
    res = run_bass_kernel_spmd(nc, in_maps, core_ids=list(range(N_CORES)),
                               trace=trace, trace_cores=trace_cores)
    y = np.empty((BATCH, OUT_F), dtype=np.float32)
    for r in range(R):
        y[r * BR:(r + 1) * BR, :] = res.results[r]["out"].T
    return y, res


def kernel(**inputs):
    y, _ = _run(inputs)
    return y

